# revision 13
# baseline (speedup 1.0000x reference)
"""Trainium2 Bass kernel for nn_EuclideanDeconf (retrieval_knn).

Computes out = -mean((x[:, :, None] - W.T[None, :, :])**2, axis=1)
            = (2*x@W.T - ||x||^2 - ||w||^2) / D

Sharding: data-parallel over batch across 8 NeuronCores (512 rows each),
W replicated. Per core:
  - x loaded fp32 (for exact ||x||^2), cast to bf16 on DVE
  - W loaded via SWDGE cast-DMA straight to bf16
  - both operands PE-transposed to d-major bf16 tiles
  - cross term as bf16 tensor-core GEMM accumulated in fp32 PSUM
  - ||w||^2/2 folded into the GEMM as one extra K=1 contraction row
  - epilogue: one ScalarE activation per tile: out = psum*(2/D) - ||x||^2/D
"""

import os

import numpy as np

B, D, C = 4096, 4096, 1024
NCORES = 8
P = 128
NW = 256  # output-tile free width (c); one PSUM bank holds 512 fp32
KG = 4    # transpose k-chunks per PSUM batch

_nc_cache = {}


def _interleave(n_x, n_w):
    """Merge x-tile and w-tile load order, x spread evenly among w."""
    items = []
    xi = wi = 0
    while xi < n_x or wi < n_w:
        if xi < n_x and (wi >= n_w or xi * n_w <= wi * n_x):
            items.append(("x", xi))
            xi += 1
        else:
            items.append(("w", wi))
            wi += 1
    return items


def _build_bass(b_sh, d, c):
    import concourse.bacc as bacc
    import concourse.mybir as mybir
    import concourse.tile as tile
    from concourse.masks import make_identity

    f32 = mybir.dt.float32
    bf16 = mybir.dt.bfloat16
    AF = mybir.ActivationFunctionType

    BT = b_sh // P    # b-tiles per core
    KC = d // P       # contraction chunks
    CT = c // P       # c-tiles
    NS = c // NW      # n-slices (output tile columns)
    CPN = NW // P     # c-tiles per n-slice
    NKG = KC // KG

    nc = bacc.Bacc(trn_type="TRN2")
    x_d = nc.dram_tensor("x", [b_sh, d], f32, kind="ExternalInput")
    w_d = nc.dram_tensor("W", [c, d], f32, kind="ExternalInput")
    o_d = nc.dram_tensor("out", [b_sh, c], f32, kind="ExternalOutput")

    with tile.TileContext(nc) as tc:
        with (
            tc.tile_pool(name="const", bufs=1) as constp,
            tc.tile_pool(name="persist", bufs=1) as persist,
            tc.tile_pool(name="xstage", bufs=4) as xstage,
            tc.tile_pool(name="wstage", bufs=5) as wstage,
            tc.tile_pool(name="bfs", bufs=3) as bfs,
            tc.tile_pool(name="outp", bufs=3) as outp,
            tc.tile_pool(name="trp", bufs=2, space="PSUM") as trp,
            tc.tile_pool(name="mmp", bufs=6, space="PSUM") as mmp,
            tc.tile_pool(name="dramp", bufs=2, space="DRAM") as dramp,
        ):
            ident = constp.tile([P, P], bf16)
            make_identity(nc, ident)
            ones_row = constp.tile([1, P], bf16)
            nc.vector.memset(ones_row, 1.0)
            wsrow = constp.tile([1, c], bf16)      # -||w_c||^2 / 2, c-major
            ws_cols = constp.tile([P, CT], f32)    # ||w||^2 per c-tile column
            ws_neg = constp.tile([P, CT], bf16)
            xs_bias = constp.tile([P, BT], f32)    # -||x_b||^2 / D per b-tile
            ws_h = constp.tile([P, 2 * CT], f32)   # per-half W square sums
            xs_h = constp.tile([P, 2 * BT], f32)   # per-half x square sums
            xT = persist.tile([P, KC, b_sh], bf16)
            wT = persist.tile([P, KC, c], bf16)
            sq_junk = persist.tile([P, d // 2], bf16)
            HD = d // 2

            def transpose_into(src_bf, dst, col0, split_copies=False,
                               kg_range=None):
                # src_bf [P, d] bf16 (rows-major) -> dst[:, :, col0:col0+P]
                for kg in (kg_range if kg_range is not None else range(NKG)):
                    pt = trp.tile([P, KG, P], bf16, tag="tr")
                    for j in range(KG):
                        kk = kg * KG + j
                        nc.tensor.transpose(
                            pt[:, j, :], src_bf[:, kk * P : (kk + 1) * P], ident
                        )
                    dst_ap = dst[:, kg * KG : (kg + 1) * KG, col0 : col0 + P]
                    if split_copies and kg % 2 == 1:
                        nc.scalar.copy(dst_ap, pt[:, :, :])
                    else:
                        nc.vector.tensor_copy(out=dst_ap, in_=pt[:, :, :])

            def do_x(bt):
                xb = bfs.tile([P, d], bf16, tag="bfs")
                for h in range(2):
                    with tc.high_priority():
                        xs = xstage.tile([P, HD], f32, tag="xs")
                        nc.sync.dma_start(
                            xs, x_d[bt * P : (bt + 1) * P, h * HD : (h + 1) * HD]
                        )
                        nc.vector.tensor_copy(
                            out=xb[:, h * HD : (h + 1) * HD], in_=xs
                        )
                        # square in place (fp32 half dead after), free-dim sum
                        nc.scalar.activation(
                            xs, xs, AF.Square,
                            accum_out=xs_h[:, 2 * bt + h : 2 * bt + h + 1],
                        )
                    transpose_into(
                        xb, xT, bt * P,
                        kg_range=range(h * NKG // 2, (h + 1) * NKG // 2),
                    )
                with tc.high_priority():
                    nc.vector.tensor_tensor(
                        xs_bias[:, bt : bt + 1],
                        xs_h[:, 2 * bt : 2 * bt + 1],
                        xs_h[:, 2 * bt + 1 : 2 * bt + 2],
                        mybir.AluOpType.add,
                    )
                    nc.vector.tensor_scalar_mul(
                        xs_bias[:, bt : bt + 1], xs_bias[:, bt : bt + 1], -1.0 / d
                    )

            def do_w(ct):
                wb = bfs.tile([P, d], bf16, tag="bfs")
                for h in range(2):
                    with tc.high_priority():
                        ws_ = wstage.tile([P, HD], f32, tag="ws")
                        nc.sync.dma_start(
                            ws_, w_d[ct * P : (ct + 1) * P, h * HD : (h + 1) * HD]
                        )
                        nc.vector.tensor_copy(
                            out=wb[:, h * HD : (h + 1) * HD], in_=ws_
                        )
                        # square from bf16; junk scratch, ws_ frees after cast
                        nc.scalar.activation(
                            sq_junk, wb[:, h * HD : (h + 1) * HD], AF.Square,
                            accum_out=ws_h[:, 2 * ct + h : 2 * ct + h + 1],
                        )
                    transpose_into(
                        wb, wT, ct * P, split_copies=True,
                        kg_range=range(h * NKG // 2, (h + 1) * NKG // 2),
                    )
                with tc.high_priority():
                    nc.vector.tensor_tensor(
                        ws_cols[:, ct : ct + 1],
                        ws_h[:, 2 * ct : 2 * ct + 1],
                        ws_h[:, 2 * ct + 1 : 2 * ct + 2],
                        mybir.AluOpType.add,
                    )

            def build_wsrow(ns):
              with tc.high_priority():
                c0 = ns * CPN
                nc.vector.tensor_scalar_mul(
                    ws_neg[:, c0 : c0 + CPN], ws_cols[:, c0 : c0 + CPN], -0.5
                )
                # cross-partition reshuffle [P, CPN] -> linear [1, NW] via DRAM
                dtmp = dramp.tile([CPN, P], bf16)
                for t in range(CPN):
                    nc.gpsimd.dma_start(dtmp[t, :], ws_neg[:, c0 + t : c0 + t + 1])
                nc.gpsimd.dma_start(wsrow[0:1, ns * NW : (ns + 1) * NW], dtmp[:, :])

            def do_mm(bt, ns):
                ps = mmp.tile([P, NW], f32, tag="mm")
                # w_sq row first so the group close never waits on wsrow
                nc.tensor.matmul(
                    ps,
                    lhsT=ones_row,
                    rhs=wsrow[0:1, ns * NW : (ns + 1) * NW],
                    start=True,
                    stop=False,
                )
                for k in range(KC):
                    nc.tensor.matmul(
                        ps,
                        lhsT=xT[:, k, bt * P : (bt + 1) * P],
                        rhs=wT[:, k, ns * NW : (ns + 1) * NW],
                        start=False,
                        stop=(k == KC - 1),
                    )
                ot = outp.tile([P, NW], f32, tag="out")
                nc.vector.tensor_scalar(
                    ot, ps, 2.0 / d, xs_bias[:, bt : bt + 1],
                    mybir.AluOpType.mult, mybir.AluOpType.add,
                )
                nc.scalar.dma_start(
                    o_d[bt * P : (bt + 1) * P, ns * NW : (ns + 1) * NW], ot
                )

            x_loaded, w_loaded, ws_built, mm_done = set(), set(), set(), set()

            def emit_ready_mms():
                for ns in range(NS):
                    if ns not in ws_built:
                        continue
                    for bt in sorted(x_loaded):
                        if (bt, ns) not in mm_done:
                            do_mm(bt, ns)
                            mm_done.add((bt, ns))

            for kind, idx in _interleave(BT, CT):
                if kind == "x":
                    do_x(idx)
                    x_loaded.add(idx)
                else:
                    do_w(idx)
                    w_loaded.add(idx)
                    for ns in range(NS):
                        if ns not in ws_built and all(
                            t in w_loaded for t in range(ns * CPN, (ns + 1) * CPN)
                        ):
                            build_wsrow(ns)
                            ws_built.add(ns)
                emit_ready_mms()
            assert len(mm_done) == BT * NS

    nc.finalize()
    return nc


def _get_nc(b_sh, d, c):
    key = (b_sh, d, c)
    if key not in _nc_cache:
        _nc_cache[key] = _build_bass(b_sh, d, c)
    return _nc_cache[key]


last_result = None


def kernel(x, W):
    global last_result
    from concourse.bass_utils import run_bass_kernel_spmd

    x = np.ascontiguousarray(x, dtype=np.float32)
    W = np.ascontiguousarray(W, dtype=np.float32)
    b_sh = x.shape[0] // NCORES
    nc = _get_nc(b_sh, x.shape[1], W.shape[0])
    in_maps = [
        {"x": np.ascontiguousarray(x[i * b_sh : (i + 1) * b_sh]), "W": W}
        for i in range(NCORES)
    ]
    kw = {}
    if os.environ.get("KERNEL_TRACE", "0") == "1":
        cores = os.environ.get("KERNEL_TRACE_CORES", "0")
        kw = dict(trace=True, trace_cores=[int(t) for t in cores.split(",")])
    res = run_bass_kernel_spmd(nc, in_maps, core_ids=list(range(NCORES)), **kw)
    last_result = res
    return np.concatenate([res.results[i]["out"] for i in range(NCORES)], axis=0)


# revision 15
# speedup vs baseline: 1.0090x; 1.0090x over previous
"""Trainium2 Bass kernel for nn_EuclideanDeconf (retrieval_knn).

Computes out = -mean((x[:, :, None] - W.T[None, :, :])**2, axis=1)
            = (2*x@W.T - ||x||^2 - ||w||^2) / D

Sharding: data-parallel over batch across 8 NeuronCores (512 rows each),
W replicated. Per core:
  - x loaded fp32 (for exact ||x||^2), cast to bf16 on DVE
  - W loaded via SWDGE cast-DMA straight to bf16
  - both operands PE-transposed to d-major bf16 tiles
  - cross term as bf16 tensor-core GEMM accumulated in fp32 PSUM
  - ||w||^2/2 folded into the GEMM as one extra K=1 contraction row
  - epilogue: one ScalarE activation per tile: out = psum*(2/D) - ||x||^2/D
"""

import os

import numpy as np

B, D, C = 4096, 4096, 1024
NCORES = 8
P = 128
NW = 256  # output-tile free width (c); one PSUM bank holds 512 fp32
KG = 4    # transpose k-chunks per PSUM batch

_nc_cache = {}


def _interleave(n_x, n_w):
    """Merge x-tile and w-tile load order, x spread evenly among w."""
    items = []
    xi = wi = 0
    while xi < n_x or wi < n_w:
        if xi < n_x and (wi >= n_w or xi * n_w <= wi * n_x):
            items.append(("x", xi))
            xi += 1
        else:
            items.append(("w", wi))
            wi += 1
    return items


def _build_bass(b_sh, d, c):
    import concourse.bacc as bacc
    import concourse.mybir as mybir
    import concourse.tile as tile
    from concourse.masks import make_identity

    f32 = mybir.dt.float32
    bf16 = mybir.dt.bfloat16
    AF = mybir.ActivationFunctionType

    BT = b_sh // P    # b-tiles per core
    KC = d // P       # contraction chunks
    CT = c // P       # c-tiles
    NS = c // NW      # n-slices (output tile columns)
    CPN = NW // P     # c-tiles per n-slice
    NKG = KC // KG

    nc = bacc.Bacc(trn_type="TRN2")
    x_d = nc.dram_tensor("x", [b_sh, d], f32, kind="ExternalInput")
    w_d = nc.dram_tensor("W", [c, d], f32, kind="ExternalInput")
    o_d = nc.dram_tensor("out", [b_sh, c], f32, kind="ExternalOutput")

    with tile.TileContext(nc) as tc:
        with (
            tc.tile_pool(name="const", bufs=1) as constp,
            tc.tile_pool(name="persist", bufs=1) as persist,
            tc.tile_pool(name="xstage", bufs=4) as xstage,
            tc.tile_pool(name="wstage", bufs=5) as wstage,
            tc.tile_pool(name="bfs", bufs=3) as bfs,
            tc.tile_pool(name="outp", bufs=3) as outp,
            tc.tile_pool(name="trp", bufs=5, space="PSUM") as trp,
            tc.tile_pool(name="mmp", bufs=3, space="PSUM") as mmp,
            tc.tile_pool(name="dramp", bufs=2, space="DRAM") as dramp,
        ):
            ident = constp.tile([P, P], bf16)
            make_identity(nc, ident)
            ones_row = constp.tile([1, P], bf16)
            nc.vector.memset(ones_row, 1.0)
            wsrow = constp.tile([1, c], bf16)      # -||w_c||^2 / 2, c-major
            ws_cols = constp.tile([P, CT], f32)    # ||w||^2 per c-tile column
            ws_neg = constp.tile([P, CT], bf16)
            xs_bias = constp.tile([P, BT], f32)    # -||x_b||^2 / D per b-tile
            ws_h = constp.tile([P, 2 * CT], f32)   # per-half W square sums
            xs_h = constp.tile([P, 2 * BT], f32)   # per-half x square sums
            xTs = [
                persist.tile([P, KC, P], bf16, name=f"xT{i}") for i in range(BT)
            ]
            wTs = [
                persist.tile([P, KC, NW], bf16, name=f"wT{i}") for i in range(NS)
            ]
            sq_junk = persist.tile([P, d // 2], bf16)
            HD = d // 2

            def transpose_into(src_bf, dst, col0, split_copies=False,
                               kg_range=None):
                # src_bf [P, d] bf16 (rows-major) -> dst[:, :, col0:col0+P]
                for kg in (kg_range if kg_range is not None else range(NKG)):
                    pt = trp.tile([P, KG, P], bf16, tag="tr")
                    for j in range(KG):
                        kk = kg * KG + j
                        nc.tensor.transpose(
                            pt[:, j, :], src_bf[:, kk * P : (kk + 1) * P], ident
                        )
                    dst_ap = dst[:, kg * KG : (kg + 1) * KG, col0 : col0 + P]
                    if split_copies and kg % 2 == 1:
                        nc.scalar.copy(dst_ap, pt[:, :, :])
                    else:
                        nc.vector.tensor_copy(out=dst_ap, in_=pt[:, :, :])

            def do_x(bt):
                xb = bfs.tile([P, d], bf16, tag="bfs")
                for h in range(2):
                    with tc.high_priority():
                        xs = xstage.tile([P, HD], f32, tag="xs")
                        nc.sync.dma_start(
                            xs, x_d[bt * P : (bt + 1) * P, h * HD : (h + 1) * HD]
                        )
                        nc.vector.tensor_copy(
                            out=xb[:, h * HD : (h + 1) * HD], in_=xs
                        )
                        # square in place (fp32 half dead after), free-dim sum
                        nc.scalar.activation(
                            xs, xs, AF.Square,
                            accum_out=xs_h[:, 2 * bt + h : 2 * bt + h + 1],
                        )
                    transpose_into(
                        xb, xTs[bt], 0,
                        kg_range=range(h * NKG // 2, (h + 1) * NKG // 2),
                    )
                with tc.high_priority():
                    nc.vector.tensor_tensor(
                        xs_bias[:, bt : bt + 1],
                        xs_h[:, 2 * bt : 2 * bt + 1],
                        xs_h[:, 2 * bt + 1 : 2 * bt + 2],
                        mybir.AluOpType.add,
                    )
                    nc.vector.tensor_scalar_mul(
                        xs_bias[:, bt : bt + 1], xs_bias[:, bt : bt + 1], -1.0 / d
                    )

            def do_w(ct):
                wb = bfs.tile([P, d], bf16, tag="bfs")
                for h in range(2):
                    with tc.high_priority():
                        ws_ = wstage.tile([P, HD], f32, tag="ws")
                        nc.sync.dma_start(
                            ws_, w_d[ct * P : (ct + 1) * P, h * HD : (h + 1) * HD]
                        )
                        nc.vector.tensor_copy(
                            out=wb[:, h * HD : (h + 1) * HD], in_=ws_
                        )
                        # square from bf16; junk scratch, ws_ frees after cast
                        nc.scalar.activation(
                            sq_junk, wb[:, h * HD : (h + 1) * HD], AF.Square,
                            accum_out=ws_h[:, 2 * ct + h : 2 * ct + h + 1],
                        )
                    transpose_into(
                        wb, wTs[ct // CPN], (ct % CPN) * P, split_copies=True,
                        kg_range=range(h * NKG // 2, (h + 1) * NKG // 2),
                    )
                with tc.high_priority():
                    nc.vector.tensor_tensor(
                        ws_cols[:, ct : ct + 1],
                        ws_h[:, 2 * ct : 2 * ct + 1],
                        ws_h[:, 2 * ct + 1 : 2 * ct + 2],
                        mybir.AluOpType.add,
                    )

            def build_wsrow(ns):
              with tc.high_priority():
                c0 = ns * CPN
                nc.vector.tensor_scalar_mul(
                    ws_neg[:, c0 : c0 + CPN], ws_cols[:, c0 : c0 + CPN], -0.5
                )
                # cross-partition reshuffle [P, CPN] -> linear [1, NW] via DRAM
                dtmp = dramp.tile([CPN, P], bf16)
                for t in range(CPN):
                    nc.gpsimd.dma_start(dtmp[t, :], ws_neg[:, c0 + t : c0 + t + 1])
                nc.gpsimd.dma_start(wsrow[0:1, ns * NW : (ns + 1) * NW], dtmp[:, :])

            def do_mm(bt, ns):
                ps = mmp.tile([P, NW], f32, tag="mm")
                # w_sq row first so the group close never waits on wsrow
                nc.tensor.matmul(
                    ps,
                    lhsT=ones_row,
                    rhs=wsrow[0:1, ns * NW : (ns + 1) * NW],
                    start=True,
                    stop=False,
                )
                for k in range(KC):
                    nc.tensor.matmul(
                        ps,
                        lhsT=xTs[bt][:, k, :],
                        rhs=wTs[ns][:, k, :],
                        start=False,
                        stop=(k == KC - 1),
                    )
                ot = outp.tile([P, NW], f32, tag="out")
                nc.vector.tensor_scalar(
                    ot, ps, 2.0 / d, xs_bias[:, bt : bt + 1],
                    mybir.AluOpType.mult, mybir.AluOpType.add,
                )
                nc.scalar.dma_start(
                    o_d[bt * P : (bt + 1) * P, ns * NW : (ns + 1) * NW], ot
                )

            x_loaded, w_loaded, ws_built, mm_done = set(), set(), set(), set()

            def emit_ready_mms():
                for ns in range(NS):
                    if ns not in ws_built:
                        continue
                    for bt in sorted(x_loaded):
                        if (bt, ns) not in mm_done:
                            do_mm(bt, ns)
                            mm_done.add((bt, ns))

            for kind, idx in _interleave(BT, CT):
                if kind == "x":
                    do_x(idx)
                    x_loaded.add(idx)
                else:
                    do_w(idx)
                    w_loaded.add(idx)
                    for ns in range(NS):
                        if ns not in ws_built and all(
                            t in w_loaded for t in range(ns * CPN, (ns + 1) * CPN)
                        ):
                            build_wsrow(ns)
                            ws_built.add(ns)
                emit_ready_mms()
            assert len(mm_done) == BT * NS

    nc.finalize()
    return nc


def _get_nc(b_sh, d, c):
    key = (b_sh, d, c)
    if key not in _nc_cache:
        _nc_cache[key] = _build_bass(b_sh, d, c)
    return _nc_cache[key]


last_result = None


def kernel(x, W):
    global last_result
    from concourse.bass_utils import run_bass_kernel_spmd

    x = np.ascontiguousarray(x, dtype=np.float32)
    W = np.ascontiguousarray(W, dtype=np.float32)
    b_sh = x.shape[0] // NCORES
    nc = _get_nc(b_sh, x.shape[1], W.shape[0])
    in_maps = [
        {"x": np.ascontiguousarray(x[i * b_sh : (i + 1) * b_sh]), "W": W}
        for i in range(NCORES)
    ]
    kw = {}
    if os.environ.get("KERNEL_TRACE", "0") == "1":
        cores = os.environ.get("KERNEL_TRACE_CORES", "0")
        kw = dict(trace=True, trace_cores=[int(t) for t in cores.split(",")])
    res = run_bass_kernel_spmd(nc, in_maps, core_ids=list(range(NCORES)), **kw)
    last_result = res
    return np.concatenate([res.results[i]["out"] for i in range(NCORES)], axis=0)


# revision 16
# speedup vs baseline: 1.0939x; 1.0842x over previous
"""Trainium2 Bass kernel for nn_EuclideanDeconf (retrieval_knn).

Computes out = -mean((x[:, :, None] - W.T[None, :, :])**2, axis=1)
            = (2*x@W.T - ||x||^2 - ||w||^2) / D

Sharding: data-parallel over batch across 8 NeuronCores (512 rows each),
W replicated. Per core:
  - x loaded fp32 (for exact ||x||^2), cast to bf16 on DVE
  - W loaded via SWDGE cast-DMA straight to bf16
  - both operands PE-transposed to d-major bf16 tiles
  - cross term as bf16 tensor-core GEMM accumulated in fp32 PSUM
  - ||w||^2/2 folded into the GEMM as one extra K=1 contraction row
  - epilogue: one ScalarE activation per tile: out = psum*(2/D) - ||x||^2/D
"""

import os

import numpy as np

B, D, C = 4096, 4096, 1024
NCORES = 8
P = 128
NW = 256  # output-tile free width (c); one PSUM bank holds 512 fp32
KG = 4    # transpose k-chunks per PSUM batch

_nc_cache = {}


def _interleave(n_x, n_w):
    """Merge x/w load order: x spread among w, but last x tile loads last
    (its matmul groups need no wsrow chain, shortening the tail)."""
    items = []
    xi = wi = 0
    while xi < n_x - 1 or wi < n_w:
        if xi < n_x - 1 and (wi >= n_w or xi * n_w <= wi * (n_x - 1)):
            items.append(("x", xi))
            xi += 1
        else:
            items.append(("w", wi))
            wi += 1
    items.append(("x", n_x - 1))
    return items


def _build_bass(b_sh, d, c):
    import concourse.bacc as bacc
    import concourse.mybir as mybir
    import concourse.tile as tile
    from concourse.masks import make_identity

    f32 = mybir.dt.float32
    bf16 = mybir.dt.bfloat16
    AF = mybir.ActivationFunctionType

    BT = b_sh // P    # b-tiles per core
    KC = d // P       # contraction chunks
    CT = c // P       # c-tiles
    NS = c // NW      # n-slices (output tile columns)
    CPN = NW // P     # c-tiles per n-slice
    NKG = KC // KG

    nc = bacc.Bacc(trn_type="TRN2")
    x_d = nc.dram_tensor("x", [b_sh, d], f32, kind="ExternalInput")
    w_d = nc.dram_tensor("W", [c, d], f32, kind="ExternalInput")
    o_d = nc.dram_tensor("out", [b_sh, c], f32, kind="ExternalOutput")

    with tile.TileContext(nc) as tc:
        with (
            tc.tile_pool(name="const", bufs=1) as constp,
            tc.tile_pool(name="persist", bufs=1) as persist,
            tc.tile_pool(name="xstage", bufs=4) as xstage,
            tc.tile_pool(name="wstage", bufs=5) as wstage,
            tc.tile_pool(name="bfs", bufs=3) as bfs,
            tc.tile_pool(name="outp", bufs=3) as outp,
            tc.tile_pool(name="trp", bufs=5, space="PSUM") as trp,
            tc.tile_pool(name="mmp", bufs=3, space="PSUM") as mmp,
            tc.tile_pool(name="dramp", bufs=2, space="DRAM") as dramp,
        ):
            ident = constp.tile([P, P], bf16)
            make_identity(nc, ident)
            ones_row = constp.tile([1, P], bf16)
            nc.vector.memset(ones_row, 1.0)
            wsrow = constp.tile([1, c], bf16)      # -||w_c||^2 / 2, c-major
            ws_cols = constp.tile([P, CT], f32)    # ||w||^2 per c-tile column
            ws_neg = constp.tile([P, CT], bf16)
            xs_bias = constp.tile([P, BT], f32)    # -||x_b||^2 / D per b-tile
            ws_h = constp.tile([P, 2 * CT], f32)   # per-half W square sums
            xs_h = constp.tile([P, 2 * BT], f32)   # per-half x square sums
            xTs = [
                persist.tile([P, KC, P], bf16, name=f"xT{i}") for i in range(BT)
            ]
            wTs = [
                persist.tile([P, KC, NW], bf16, name=f"wT{i}") for i in range(NS)
            ]
            sq_junk = persist.tile([P, d // 2], bf16)
            HD = d // 2

            def transpose_into(src_bf, dst, col0, split_copies=False,
                               kg_range=None):
                # src_bf [P, d] bf16 (rows-major) -> dst[:, :, col0:col0+P]
                for kg in (kg_range if kg_range is not None else range(NKG)):
                    pt = trp.tile([P, KG, P], bf16, tag="tr")
                    for j in range(KG):
                        kk = kg * KG + j
                        nc.tensor.transpose(
                            pt[:, j, :], src_bf[:, kk * P : (kk + 1) * P], ident
                        )
                    dst_ap = dst[:, kg * KG : (kg + 1) * KG, col0 : col0 + P]
                    if split_copies and kg % 2 == 1:
                        nc.scalar.copy(dst_ap, pt[:, :, :])
                    else:
                        nc.vector.tensor_copy(out=dst_ap, in_=pt[:, :, :])

            def do_x(bt):
                xb = bfs.tile([P, d], bf16, tag="bfs")
                for h in range(2):
                    with tc.high_priority():
                        xs = xstage.tile([P, HD], f32, tag="xs")
                        nc.sync.dma_start(
                            xs, x_d[bt * P : (bt + 1) * P, h * HD : (h + 1) * HD]
                        )
                        nc.vector.tensor_copy(
                            out=xb[:, h * HD : (h + 1) * HD], in_=xs
                        )
                        # square in place (fp32 half dead after), free-dim sum
                        nc.scalar.activation(
                            xs, xs, AF.Square,
                            accum_out=xs_h[:, 2 * bt + h : 2 * bt + h + 1],
                        )
                    transpose_into(
                        xb, xTs[bt], 0,
                        kg_range=range(h * NKG // 2, (h + 1) * NKG // 2),
                    )
                with tc.high_priority():
                    nc.vector.tensor_tensor(
                        xs_bias[:, bt : bt + 1],
                        xs_h[:, 2 * bt : 2 * bt + 1],
                        xs_h[:, 2 * bt + 1 : 2 * bt + 2],
                        mybir.AluOpType.add,
                    )
                    nc.vector.tensor_scalar_mul(
                        xs_bias[:, bt : bt + 1], xs_bias[:, bt : bt + 1], -1.0 / d
                    )

            def do_w(ct):
                wb = bfs.tile([P, d], bf16, tag="bfs")
                for h in range(2):
                    with tc.high_priority():
                        ws_ = wstage.tile([P, HD], f32, tag="ws")
                        nc.sync.dma_start(
                            ws_, w_d[ct * P : (ct + 1) * P, h * HD : (h + 1) * HD]
                        )
                        nc.vector.tensor_copy(
                            out=wb[:, h * HD : (h + 1) * HD], in_=ws_
                        )
                        # square from bf16; junk scratch, ws_ frees after cast
                        nc.scalar.activation(
                            sq_junk, wb[:, h * HD : (h + 1) * HD], AF.Square,
                            accum_out=ws_h[:, 2 * ct + h : 2 * ct + h + 1],
                        )
                    transpose_into(
                        wb, wTs[ct // CPN], (ct % CPN) * P, split_copies=True,
                        kg_range=range(h * NKG // 2, (h + 1) * NKG // 2),
                    )
                with tc.high_priority():
                    nc.vector.tensor_tensor(
                        ws_cols[:, ct : ct + 1],
                        ws_h[:, 2 * ct : 2 * ct + 1],
                        ws_h[:, 2 * ct + 1 : 2 * ct + 2],
                        mybir.AluOpType.add,
                    )

            def build_wsrow(ns):
              with tc.high_priority():
                c0 = ns * CPN
                nc.vector.tensor_scalar_mul(
                    ws_neg[:, c0 : c0 + CPN], ws_cols[:, c0 : c0 + CPN], -0.5
                )
                # cross-partition reshuffle [P, CPN] -> linear [1, NW] via DRAM
                dtmp = dramp.tile([CPN, P], bf16)
                for t in range(CPN):
                    nc.gpsimd.dma_start(dtmp[t, :], ws_neg[:, c0 + t : c0 + t + 1])
                nc.gpsimd.dma_start(wsrow[0:1, ns * NW : (ns + 1) * NW], dtmp[:, :])

            def do_mm(bt, ns):
                ps = mmp.tile([P, NW], f32, tag="mm")
                # w_sq row first so the group close never waits on wsrow
                nc.tensor.matmul(
                    ps,
                    lhsT=ones_row,
                    rhs=wsrow[0:1, ns * NW : (ns + 1) * NW],
                    start=True,
                    stop=False,
                )
                for k in range(KC):
                    nc.tensor.matmul(
                        ps,
                        lhsT=xTs[bt][:, k, :],
                        rhs=wTs[ns][:, k, :],
                        start=False,
                        stop=(k == KC - 1),
                    )
                ot = outp.tile([P, NW], f32, tag="out")
                nc.vector.tensor_scalar(
                    ot, ps, 2.0 / d, xs_bias[:, bt : bt + 1],
                    mybir.AluOpType.mult, mybir.AluOpType.add,
                )
                nc.scalar.dma_start(
                    o_d[bt * P : (bt + 1) * P, ns * NW : (ns + 1) * NW], ot
                )

            x_loaded, w_loaded, ws_built, mm_done = set(), set(), set(), set()

            def emit_ready_mms():
                for ns in range(NS):
                    if ns not in ws_built:
                        continue
                    for bt in sorted(x_loaded):
                        if (bt, ns) not in mm_done:
                            do_mm(bt, ns)
                            mm_done.add((bt, ns))

            for kind, idx in _interleave(BT, CT):
                if kind == "x":
                    do_x(idx)
                    x_loaded.add(idx)
                else:
                    do_w(idx)
                    w_loaded.add(idx)
                    for ns in range(NS):
                        if ns not in ws_built and all(
                            t in w_loaded for t in range(ns * CPN, (ns + 1) * CPN)
                        ):
                            build_wsrow(ns)
                            ws_built.add(ns)
                emit_ready_mms()
            assert len(mm_done) == BT * NS

    nc.finalize()
    return nc


def _get_nc(b_sh, d, c):
    key = (b_sh, d, c)
    if key not in _nc_cache:
        _nc_cache[key] = _build_bass(b_sh, d, c)
    return _nc_cache[key]


last_result = None


def kernel(x, W):
    global last_result
    from concourse.bass_utils import run_bass_kernel_spmd

    x = np.ascontiguousarray(x, dtype=np.float32)
    W = np.ascontiguousarray(W, dtype=np.float32)
    b_sh = x.shape[0] // NCORES
    nc = _get_nc(b_sh, x.shape[1], W.shape[0])
    in_maps = [
        {"x": np.ascontiguousarray(x[i * b_sh : (i + 1) * b_sh]), "W": W}
        for i in range(NCORES)
    ]
    kw = {}
    if os.environ.get("KERNEL_TRACE", "0") == "1":
        cores = os.environ.get("KERNEL_TRACE_CORES", "0")
        kw = dict(trace=True, trace_cores=[int(t) for t in cores.split(",")])
    res = run_bass_kernel_spmd(nc, in_maps, core_ids=list(range(NCORES)), **kw)
    last_result = res
    return np.concatenate([res.results[i]["out"] for i in range(NCORES)], axis=0)


# revision 18
# speedup vs baseline: 1.2322x; 1.1264x over previous
"""Trainium2 Bass kernel for nn_EuclideanDeconf (retrieval_knn).

Computes out = -mean((x[:, :, None] - W.T[None, :, :])**2, axis=1)
            = (2*x@W.T - ||x||^2 - ||w||^2) / D

Sharding: data-parallel over batch across 8 NeuronCores (512 rows each),
W replicated. Per core:
  - x loaded fp32 (for exact ||x||^2), cast to bf16 on DVE
  - W loaded via SWDGE cast-DMA straight to bf16
  - both operands PE-transposed to d-major bf16 tiles
  - cross term as bf16 tensor-core GEMM accumulated in fp32 PSUM
  - ||w||^2/2 folded into the GEMM as one extra K=1 contraction row
  - epilogue: one ScalarE activation per tile: out = psum*(2/D) - ||x||^2/D
"""

import os

import numpy as np

B, D, C = 4096, 4096, 1024
NCORES = 8
P = 128
NW = 256  # output-tile free width (c); one PSUM bank holds 512 fp32
KG = 4    # transpose k-chunks per PSUM batch

_nc_cache = {}


def _interleave(n_x, n_w):
    """Merge x/w load order: x spread among w, but last x tile loads last
    (its matmul groups need no wsrow chain, shortening the tail)."""
    items = []
    xi = wi = 0
    while xi < n_x - 1 or wi < n_w:
        if xi < n_x - 1 and (wi >= n_w or xi * n_w <= wi * (n_x - 1)):
            items.append(("x", xi))
            xi += 1
        else:
            items.append(("w", wi))
            wi += 1
    items.append(("x", n_x - 1))
    return items


def _build_bass(b_sh, d, c):
    import concourse.bacc as bacc
    import concourse.mybir as mybir
    import concourse.tile as tile
    from concourse.masks import make_identity

    f32 = mybir.dt.float32
    bf16 = mybir.dt.bfloat16
    AF = mybir.ActivationFunctionType

    BT = b_sh // P    # b-tiles per core
    KC = d // P       # contraction chunks
    CT = c // P       # c-tiles
    NS = c // NW      # n-slices (output tile columns)
    CPN = NW // P     # c-tiles per n-slice
    NKG = KC // KG

    nc = bacc.Bacc(trn_type="TRN2")
    x_d = nc.dram_tensor("x", [b_sh, d], f32, kind="ExternalInput")
    w_d = nc.dram_tensor("W", [c, d], f32, kind="ExternalInput")
    o_d = nc.dram_tensor("out", [b_sh, c], f32, kind="ExternalOutput")

    with tile.TileContext(nc) as tc:
        with (
            tc.tile_pool(name="const", bufs=1) as constp,
            tc.tile_pool(name="persist", bufs=1) as persist,
            tc.tile_pool(name="xstage", bufs=4) as xstage,
            tc.tile_pool(name="wstage", bufs=5) as wstage,
            tc.tile_pool(name="bfs", bufs=3) as bfs,
            tc.tile_pool(name="outp", bufs=3) as outp,
            tc.tile_pool(name="trp", bufs=5, space="PSUM") as trp,
            tc.tile_pool(name="mmp", bufs=3, space="PSUM") as mmp,
            tc.tile_pool(name="dramp", bufs=2, space="DRAM") as dramp,
        ):
            ident = constp.tile([P, P], bf16)
            make_identity(nc, ident)
            wsrow = constp.tile([1, c], f32)       # -||w_c||^2 / D, c-major
            ws_rep = persist.tile([P, c], f32)     # wsrow broadcast to 128 p
            ws_cols = constp.tile([P, CT], f32)    # ||w||^2 per c-tile column
            ws_neg = constp.tile([P, CT], f32)
            xs_bias = constp.tile([P, BT], f32)    # -||x_b||^2 / D per b-tile
            ws_h = constp.tile([P, 2 * CT], f32)   # per-half W square sums
            xs_h = constp.tile([P, 2 * BT], f32)   # per-half x square sums
            xTs = [
                persist.tile([P, KC, P], bf16, name=f"xT{i}") for i in range(BT)
            ]
            wTs = [
                persist.tile([P, KC, NW], bf16, name=f"wT{i}") for i in range(NS)
            ]
            sq_junk = persist.tile([P, d // 2], bf16)
            HD = d // 2

            def transpose_into(src_bf, dst, col0, split_copies=False,
                               kg_range=None):
                # src_bf [P, d] bf16 (rows-major) -> dst[:, :, col0:col0+P]
                for kg in (kg_range if kg_range is not None else range(NKG)):
                    pt = trp.tile([P, KG, P], bf16, tag="tr")
                    for j in range(KG):
                        kk = kg * KG + j
                        nc.tensor.transpose(
                            pt[:, j, :], src_bf[:, kk * P : (kk + 1) * P], ident
                        )
                    dst_ap = dst[:, kg * KG : (kg + 1) * KG, col0 : col0 + P]
                    if split_copies and kg % 2 == 1:
                        nc.scalar.copy(dst_ap, pt[:, :, :])
                    else:
                        nc.vector.tensor_copy(out=dst_ap, in_=pt[:, :, :])

            def do_x(bt):
                xb = bfs.tile([P, d], bf16, tag="bfs")
                for h in range(2):
                    with tc.high_priority():
                        xs = xstage.tile([P, HD], f32, tag="xs")
                        nc.sync.dma_start(
                            xs, x_d[bt * P : (bt + 1) * P, h * HD : (h + 1) * HD]
                        )
                        nc.vector.tensor_copy(
                            out=xb[:, h * HD : (h + 1) * HD], in_=xs
                        )
                        # square in place (fp32 half dead after), free-dim sum
                        nc.scalar.activation(
                            xs, xs, AF.Square,
                            accum_out=xs_h[:, 2 * bt + h : 2 * bt + h + 1],
                        )
                    transpose_into(
                        xb, xTs[bt], 0,
                        kg_range=range(h * NKG // 2, (h + 1) * NKG // 2),
                    )
                with tc.high_priority():
                    nc.vector.tensor_tensor(
                        xs_bias[:, bt : bt + 1],
                        xs_h[:, 2 * bt : 2 * bt + 1],
                        xs_h[:, 2 * bt + 1 : 2 * bt + 2],
                        mybir.AluOpType.add,
                    )
                    nc.vector.tensor_scalar_mul(
                        xs_bias[:, bt : bt + 1], xs_bias[:, bt : bt + 1], -1.0 / d
                    )

            def do_w(ct):
                wb = bfs.tile([P, d], bf16, tag="bfs")
                for h in range(2):
                    with tc.high_priority():
                        ws_ = wstage.tile([P, HD], f32, tag="ws")
                        nc.gpsimd.dma_start(
                            ws_, w_d[ct * P : (ct + 1) * P, h * HD : (h + 1) * HD]
                        )
                        nc.vector.tensor_copy(
                            out=wb[:, h * HD : (h + 1) * HD], in_=ws_
                        )
                        # square from bf16; junk scratch, ws_ frees after cast
                        nc.scalar.activation(
                            sq_junk, wb[:, h * HD : (h + 1) * HD], AF.Square,
                            accum_out=ws_h[:, 2 * ct + h : 2 * ct + h + 1],
                        )
                    transpose_into(
                        wb, wTs[ct // CPN], (ct % CPN) * P, split_copies=True,
                        kg_range=range(h * NKG // 2, (h + 1) * NKG // 2),
                    )
                with tc.high_priority():
                    nc.vector.tensor_tensor(
                        ws_cols[:, ct : ct + 1],
                        ws_h[:, 2 * ct : 2 * ct + 1],
                        ws_h[:, 2 * ct + 1 : 2 * ct + 2],
                        mybir.AluOpType.add,
                    )

            def build_wsrow(ns):
              with tc.high_priority():
                c0 = ns * CPN
                nc.vector.tensor_scalar_mul(
                    ws_neg[:, c0 : c0 + CPN], ws_cols[:, c0 : c0 + CPN], -1.0 / d
                )
                # cross-partition reshuffle [P, CPN] -> linear [1, NW] via DRAM
                dtmp = dramp.tile([CPN, P], f32)
                for t in range(CPN):
                    nc.gpsimd.dma_start(dtmp[t, :], ws_neg[:, c0 + t : c0 + t + 1])
                nc.gpsimd.dma_start(wsrow[0:1, ns * NW : (ns + 1) * NW], dtmp[:, :])
                nc.gpsimd.partition_broadcast(
                    ws_rep[:, ns * NW : (ns + 1) * NW],
                    wsrow[0:1, ns * NW : (ns + 1) * NW],
                )

            def do_mm(bt, ns):
                ps = mmp.tile([P, NW], f32, tag="mm")
                for k in range(KC):
                    nc.tensor.matmul(
                        ps,
                        lhsT=xTs[bt][:, k, :],
                        rhs=wTs[ns][:, k, :],
                        start=(k == 0),
                        stop=(k == KC - 1),
                    )
                ot = outp.tile([P, NW], f32, tag="out")
                nc.vector.tensor_scalar(
                    ot, ps, 2.0 / d, xs_bias[:, bt : bt + 1],
                    mybir.AluOpType.mult, mybir.AluOpType.add,
                )
                nc.vector.tensor_tensor(
                    ot, ot, ws_rep[:, ns * NW : (ns + 1) * NW],
                    mybir.AluOpType.add,
                )
                nc.scalar.dma_start(
                    o_d[bt * P : (bt + 1) * P, ns * NW : (ns + 1) * NW], ot
                )

            x_loaded, w_loaded, ws_built, mm_done = set(), set(), set(), set()

            def emit_ready_mms():
                for ns in range(NS):
                    if ns not in ws_built:
                        continue
                    for bt in sorted(x_loaded):
                        if (bt, ns) not in mm_done:
                            do_mm(bt, ns)
                            mm_done.add((bt, ns))

            for kind, idx in _interleave(BT, CT):
                if kind == "x":
                    do_x(idx)
                    x_loaded.add(idx)
                else:
                    do_w(idx)
                    w_loaded.add(idx)
                    for ns in range(NS):
                        if ns not in ws_built and all(
                            t in w_loaded for t in range(ns * CPN, (ns + 1) * CPN)
                        ):
                            build_wsrow(ns)
                            ws_built.add(ns)
                emit_ready_mms()
            assert len(mm_done) == BT * NS

    nc.finalize()
    return nc


def _get_nc(b_sh, d, c):
    key = (b_sh, d, c)
    if key not in _nc_cache:
        _nc_cache[key] = _build_bass(b_sh, d, c)
    return _nc_cache[key]


last_result = None


def kernel(x, W):
    global last_result
    from concourse.bass_utils import run_bass_kernel_spmd

    x = np.ascontiguousarray(x, dtype=np.float32)
    W = np.ascontiguousarray(W, dtype=np.float32)
    b_sh = x.shape[0] // NCORES
    nc = _get_nc(b_sh, x.shape[1], W.shape[0])
    in_maps = [
        {"x": np.ascontiguousarray(x[i * b_sh : (i + 1) * b_sh]), "W": W}
        for i in range(NCORES)
    ]
    kw = {}
    if os.environ.get("KERNEL_TRACE", "0") == "1":
        cores = os.environ.get("KERNEL_TRACE_CORES", "0")
        kw = dict(trace=True, trace_cores=[int(t) for t in cores.split(",")])
    res = run_bass_kernel_spmd(nc, in_maps, core_ids=list(range(NCORES)), **kw)
    last_result = res
    return np.concatenate([res.results[i]["out"] for i in range(NCORES)], axis=0)
